# revision 13
# baseline (speedup 1.0000x reference)
"""Multi-head relative-position attention on 8 trn2 NeuronCores.

Sharding: head-parallel. Each core owns 2 of the 16 heads (all 4 batches).
 - Wq/Wk/Wv column-sliced, Wo row-sliced per core; x replicated (transposed).
 - Relative term: rel[q,k] = Q[q] . table[clip(q-k,-128,128)+128].
   Per q-tile we matmul Q against a padded+reversed table G -> D rows in DRAM,
   then re-read D with a diagonal (skewed) access pattern per near-diagonal
   (q,k) tile and PE-transpose-accumulate into the scores PSUM.
   Tiles fully outside the +-128 band use a constant table row, folded into
   the content matmul as K+T0 / K+T256.
 - Scores are computed transposed [k, q]; softmax denom comes from a ones
   column appended to V in the attn@V matmul; normalization applied to the
   per-head output (per-column broadcast via gpsimd partition_broadcast).
 - Host sums the 8 partial Wo products and adds bo.
"""

import os
from contextlib import ExitStack

import numpy as np

import concourse.bass as bass
import concourse.mybir as mybir
import concourse.tile as tile
from concourse import bacc
from concourse.bass_utils import run_bass_kernel_spmd
from concourse.masks import make_identity

DEPTH = 64
NUM_HEADS = 16
DM = 1024
MAX_REL = 128
B, S = 4, 1024
NCORES = 8
HL = NUM_HEADS // NCORES  # heads per core = 2
TOK = B * S

F32 = mybir.dt.float32
F32R = mybir.dt.float32r
BF16 = mybir.dt.bfloat16

# matmul-input dtype: float32r = full-rate fp32 matmul (walrus requires
# matmul inputs to be declared/produced as f32r)
MM_DT = F32R

TRACE = False
LAST = {}

_built = None


def _mm(ap):
    return ap.bitcast(MM_DT) if MM_DT is not F32 else ap


ALU_ADD = mybir.AluOpType.add


def _runs(labels):
    """group consecutive equal labels -> list of (label, start, len)"""
    out = []
    for i, l in enumerate(labels):
        if out and out[-1][0] == l:
            out[-1][2] += 1
        else:
            out.append([l, i, 1])
    return [tuple(r) for r in out]


def _build():
    global _built
    if _built is not None:
        return _built

    nc = bacc.Bacc("TRN2", target_bir_lowering=False, debug=False,
                   num_devices=NCORES)

    xT = nc.dram_tensor("xT", [DM, TOK], F32, kind="ExternalInput").ap()
    wq = nc.dram_tensor("wq", [DM, 128], F32, kind="ExternalInput").ap()
    wk = nc.dram_tensor("wk", [DM, 128], F32, kind="ExternalInput").ap()
    wv = nc.dram_tensor("wv", [DM, 128], F32, kind="ExternalInput").ap()
    bqkv = nc.dram_tensor("bqkv", [128, 3], F32, kind="ExternalInput").ap()
    g2 = nc.dram_tensor("g2", [128, 512], F32, kind="ExternalInput").ap()
    tcols = nc.dram_tensor("tcols", [128, 2], F32, kind="ExternalInput").ap()
    wo = nc.dram_tensor("wo", [128, DM], F32, kind="ExternalInput").ap()
    partial = nc.dram_tensor("partial", [TOK, DM], F32,
                             kind="ExternalOutput").ap()
    # skew buffer: one [S, 512] f32 region per (b, local h)
    dbuf = nc.dram_tensor("dbuf", [B * HL * S * 512], F32)

    with tile.TileContext(nc) as tc, ExitStack() as ctx:
        singles = ctx.enter_context(tc.tile_pool(name="singles", bufs=1))
        xpool = ctx.enter_context(tc.tile_pool(name="xp", bufs=1))
        projp = ctx.enter_context(tc.tile_pool(name="projp", bufs=2))
        kvarp = ctx.enter_context(tc.tile_pool(name="kvarp", bufs=2))
        vnp = ctx.enter_context(tc.tile_pool(name="vnp", bufs=4))
        expp = ctx.enter_context(tc.tile_pool(name="expp", bufs=4))
        ohp = ctx.enter_context(tc.tile_pool(name="ohp", bufs=2))
        drp = ctx.enter_context(tc.tile_pool(name="drp", bufs=8))
        dsp = ctx.enter_context(tc.tile_pool(name="dsp", bufs=3))
        wop = ctx.enter_context(tc.tile_pool(name="wop", bufs=3))
        recp = ctx.enter_context(tc.tile_pool(name="recp", bufs=4))
        ppsum = ctx.enter_context(tc.tile_pool(name="ppsum", bufs=2, space="PSUM"))
        spsum = ctx.enter_context(tc.tile_pool(name="spsum", bufs=2, space="PSUM"))
        vpsum = ctx.enter_context(tc.tile_pool(name="vpsum", bufs=1, space="PSUM"))
        apsum = ctx.enter_context(tc.tile_pool(name="apsum", bufs=3, space="PSUM"))

        w_sb = {}
        for name, dram in (("wq", wq), ("wk", wk), ("wv", wv)):
            t = singles.tile([128, 8, 128], MM_DT, tag=f"w_{name}")
            nc.sync.dma_start(t[:], _mm(dram.rearrange("(c p) m -> p c m", p=128)))
            w_sb[name] = t
        wo_sb = singles.tile([128, DM], MM_DT, tag="wo")
        nc.sync.dma_start(wo_sb[:], _mm(wo))
        g_sb = singles.tile([128, 512], MM_DT, tag="g")
        nc.sync.dma_start(g_sb[:], _mm(g2))
        tc_sb = singles.tile([128, 2], F32, tag="tc")
        nc.sync.dma_start(tc_sb[:], tcols)
        b_sb = singles.tile([128, 3], F32, tag="b")
        nc.sync.dma_start(b_sb[:], bqkv)
        idb = singles.tile([128, 128], BF16, tag="idb")
        make_identity(nc, idb[:])
        idf = singles.tile([128, 128], F32, tag="idf")
        make_identity(nc, idf[:])

        xT_r = xT.rearrange("(c p) t -> p c t", p=128)  # [128, 8, TOK]

        for b in range(B):
            x_sb = xpool.tile([128, 8, S], MM_DT, tag="x")
            nc.sync.dma_start(x_sb[:], _mm(xT_r[:, :, b * S:(b + 1) * S]))

            qt = projp.tile([128, S], MM_DT, tag="qt")
            kt = projp.tile([128, S], MM_DT, tag="kt")
            vt = projp.tile([128, S], BF16, tag="vt")
            # K + T0 / K + T256 variants (rel term outside the +-128 band)
            k0 = kvarp.tile([128, S], MM_DT, tag="k0")
            k256 = kvarp.tile([128, S], MM_DT, tag="k256")
            for pi, (wname, psb) in enumerate(
                    (("wq", qt), ("wk", kt), ("wv", vt))):
                for nt in range(2):
                    ns = slice(nt * 512, (nt + 1) * 512)
                    ps = ppsum.tile([128, 512], F32, tag="ps")
                    for kc in range(8):
                        nc.tensor.matmul(
                            ps[:],
                            w_sb[wname][:, kc, :],
                            x_sb[:, kc, nt * 512:(nt + 1) * 512],
                            start=(kc == 0), stop=(kc == 7))
                    nc.vector.tensor_scalar_add(
                        psb[:, ns], ps[:], b_sb[:, pi:pi + 1])
                    if wname == "wk":
                        nc.vector.tensor_scalar(
                            k0[:, ns], ps[:], b_sb[:, pi:pi + 1],
                            tc_sb[:, 0:1], ALU_ADD, ALU_ADD)
                        nc.vector.tensor_scalar(
                            k256[:, ns], ps[:], b_sb[:, pi:pi + 1],
                            tc_sb[:, 1:2], ALU_ADD, ALU_ADD)

            # V in natural layout [k, dv] via PE transpose (bf16), + ones col
            vns = []
            for h in range(HL):
                hs = slice(h * 64, (h + 1) * 64)
                vn = vnp.tile([128, 8, 66], BF16, tag="vn")
                nc.vector.memset(vn[:, :, 64:65], 1.0)
                nc.vector.memset(vn[:, :, 65:66], 0.0)
                for k2 in range(8):
                    vps = vpsum.tile([128, 64], BF16, tag="vps")
                    nc.tensor.matmul(
                        vps[:], vt[hs, k2 * 128:(k2 + 1) * 128],
                        idb[hs, hs], is_transpose=True, start=True, stop=True)
                    nc.vector.tensor_copy(vn[:, k2, 0:64], vps[:])
                vns.append(vn)

            ohT = ohp.tile([128, S], MM_DT, tag="oh")

            # --- R phase: D rows = Q . G -> DRAM skew buffer (bf16)
            for q2 in range(8):
                rpss = []
                for h in range(HL):
                    hs = slice(h * 64, (h + 1) * 64)
                    rps = ppsum.tile([128, 512], F32, tag="ps")
                    nc.tensor.matmul(
                        rps[:],
                        qt[hs, q2 * 128:(q2 + 1) * 128],
                        g_sb[hs, :], start=True, stop=True)
                    rpss.append(rps)
                for h in range(HL):
                    bh = b * HL + h
                    dsb = dsp.tile([128, 512], F32, tag="ds")
                    nc.scalar.copy(dsb[:], rpss[h][:])
                    nc.sync.dma_start(
                        bass.AP(tensor=dbuf,
                                offset=bh * S * 512 + q2 * 128 * 512,
                                ap=[[512, 128], [1, 512]]),
                        dsb[:])

            # --- scores (transposed) + softmax + attn@V per q-half
            for qh in range(2):
                apss = [apsum.tile([66, 512], F32, tag="aps", name=f"aps{h}")
                        for h in range(HL)]
                exs = [expp.tile([128, 8, 512], BF16, tag="ex",
                                 name=f"ex{h}")
                       for h in range(HL)]
                def _cls(k2):
                    return ["f256" if (qh * 512 + qs * 128) - k2 * 128 >= 256
                            else ("f0" if (qh * 512 + qs * 128) - k2 * 128
                                  <= -256 else "near")
                            for qs in range(4)]
                k2_order = sorted(range(8),
                                  key=lambda k2: "near" in _cls(k2))
                for k2 in k2_order:
                    cls = _cls(k2)
                    runs = _runs(cls)
                    n_near = sum(1 for c in cls if c == "near")
                    n_mm = len(runs) + n_near
                    spss = []
                    for h in range(HL):
                        hs = slice(h * 64, (h + 1) * 64)
                        sps = spsum.tile([128, 512], F32, tag="sps",
                                         name=f"sps{h}")
                        for mi, (cl, qs0, ln) in enumerate(runs):
                            lhs = {"near": kt, "f0": k0, "f256": k256}[cl]
                            n0 = qs0 * 128
                            nc.tensor.matmul(
                                sps[:, n0:n0 + ln * 128],
                                lhs[hs, k2 * 128:(k2 + 1) * 128],
                                qt[hs, qh * 512 + n0:
                                   qh * 512 + n0 + ln * 128],
                                start=(mi == 0), stop=(mi == n_mm - 1))
                        spss.append(sps)
                    for h in range(HL):
                        bh = b * HL + h
                        mi = len(runs)
                        for qs in range(4):
                            if cls[qs] != "near":
                                continue
                            q0a = qh * 512 + qs * 128
                            off = (bh * S * 512 + q0a * 512
                                   + (k2 * 128 - q0a + 255))
                            dr = drp.tile([128, 128], F32, tag="dr")
                            nc.sync.dma_start(
                                dr[:],
                                bass.AP(tensor=dbuf, offset=off,
                                        ap=[[511, 128], [1, 128]]))
                            nc.tensor.matmul(
                                spss[h][:, qs * 128:(qs + 1) * 128],
                                dr[:], idf[:],
                                is_transpose=True, start=(mi == 0),
                                stop=(mi == n_mm - 1))
                            mi += 1
                    for h in range(HL):
                        nc.scalar.activation(
                            exs[h][:, k2, :], spss[h][:],
                            mybir.ActivationFunctionType.Exp, scale=0.125)
                for h in range(HL):
                    for k2 in range(8):
                        nc.tensor.matmul(
                            apss[h][:], vns[h][:, k2, 0:66],
                            exs[h][:, k2, :],
                            start=(k2 == 0), stop=(k2 == 7))
                for h in range(HL):
                    hs = slice(h * 64, (h + 1) * 64)
                    rec = recp.tile([1, 512], F32, tag="rec")
                    nc.vector.reciprocal(rec[:], apss[h][64:65, :])
                    rbc = recp.tile([64, 512], F32, tag="rbc")
                    nc.gpsimd.partition_broadcast(rbc[:], rec[:])
                    nc.vector.tensor_mul(
                        ohT[hs, qh * 512:(qh + 1) * 512],
                        apss[h][0:64, :], rbc[:])

            # --- output projection (partial: this core's 2 heads only)
            for tt in range(8):
                for nt in range(2):
                    ops = ppsum.tile([128, 512], F32, tag="ps")
                    nc.tensor.matmul(
                        ops[:], ohT[:, tt * 128:(tt + 1) * 128],
                        wo_sb[:, nt * 512:(nt + 1) * 512],
                        start=True, stop=True)
                    osb = wop.tile([128, 512], F32, tag="os")
                    nc.vector.tensor_copy(osb[:], ops[:])
                    nc.sync.dma_start(
                        partial[b * S + tt * 128:b * S + (tt + 1) * 128,
                                nt * 512:(nt + 1) * 512], osb[:])

    nc.compile()
    _built = nc
    return nc


def kernel(x, Wq, bq, Wk, bk, Wv, bv, Wo, bo, rel_table):
    x = np.asarray(x, np.float32)
    Wq = np.asarray(Wq, np.float32)
    Wk = np.asarray(Wk, np.float32)
    Wv = np.asarray(Wv, np.float32)
    Wo = np.asarray(Wo, np.float32)
    bq = np.asarray(bq, np.float32)
    bk = np.asarray(bk, np.float32)
    bv = np.asarray(bv, np.float32)
    bo = np.asarray(bo, np.float32)
    rel_table = np.asarray(rel_table, np.float32)

    xT = np.ascontiguousarray(x.reshape(TOK, DM).T)  # [DM, TOK]
    # G[d, m'] = table[clip(255 - m', -128, 128) + 128, d], m' in [0, 511)
    idx = np.clip(255 - np.arange(511), -MAX_REL, MAX_REL) + MAX_REL
    G = np.concatenate([rel_table[idx].T,
                        np.zeros((DEPTH, 1), np.float32)], axis=1)  # [64, 512]
    g2 = np.ascontiguousarray(np.concatenate([G, G], axis=0))  # [128, 512]
    t0 = np.concatenate([rel_table[0], rel_table[0]])
    t256 = np.concatenate([rel_table[2 * MAX_REL], rel_table[2 * MAX_REL]])
    tcols = np.ascontiguousarray(np.stack([t0, t256], axis=1))  # [128, 2]

    in_maps = []
    for c in range(NCORES):
        sl = slice(c * 128, (c + 1) * 128)
        in_maps.append({
            "xT": xT,
            "wq": np.ascontiguousarray(Wq[:, sl]),
            "wk": np.ascontiguousarray(Wk[:, sl]),
            "wv": np.ascontiguousarray(Wv[:, sl]),
            "bqkv": np.ascontiguousarray(
                np.stack([bq[sl], bk[sl], bv[sl]], axis=1)),
            "g2": g2,
            "tcols": tcols,
            "wo": np.ascontiguousarray(Wo[sl, :]),
        })

    nc = _build()
    res = run_bass_kernel_spmd(nc, in_maps, list(range(NCORES)),
                               trace=TRACE or bool(os.environ.get("KTRACE")))
    LAST["results"] = res
    out = res.results[0]["partial"].astype(np.float64)
    for c in range(1, NCORES):
        out += res.results[c]["partial"]
    out = (out + bo.astype(np.float64)).astype(np.float32)
    return out.reshape(B, S, DM)


# revision 15
# speedup vs baseline: 1.0532x; 1.0532x over previous
"""Multi-head relative-position attention on 8 trn2 NeuronCores.

Sharding: head-parallel. Each core owns 2 of the 16 heads (all 4 batches).
 - Wq/Wk/Wv column-sliced, Wo row-sliced per core; x replicated (transposed).
 - Relative term: rel[q,k] = Q[q] . table[clip(q-k,-128,128)+128].
   Per q-tile we matmul Q against a padded+reversed table G -> D rows in DRAM,
   then re-read D with a diagonal (skewed) access pattern per near-diagonal
   (q,k) tile and PE-transpose-accumulate into the scores PSUM.
   Tiles fully outside the +-128 band use a constant table row, folded into
   the content matmul as K+T0 / K+T256.
 - Scores are computed transposed [k, q]; softmax denom comes from a ones
   column appended to V in the attn@V matmul; normalization applied to the
   per-head output (per-column broadcast via gpsimd partition_broadcast).
 - Host sums the 8 partial Wo products and adds bo.
"""

import os
from contextlib import ExitStack

import numpy as np

import concourse.bass as bass
import concourse.mybir as mybir
import concourse.tile as tile
from concourse import bacc
from concourse.bass_utils import run_bass_kernel_spmd
from concourse.masks import make_identity

DEPTH = 64
NUM_HEADS = 16
DM = 1024
MAX_REL = 128
B, S = 4, 1024
NCORES = 8
HL = NUM_HEADS // NCORES  # heads per core = 2
TOK = B * S

F32 = mybir.dt.float32
F32R = mybir.dt.float32r
BF16 = mybir.dt.bfloat16

# matmul-input dtype: float32r = full-rate fp32 matmul (walrus requires
# matmul inputs to be declared/produced as f32r)
MM_DT = F32R

TRACE = False
LAST = {}

_built = None


def _mm(ap):
    return ap.bitcast(MM_DT) if MM_DT is not F32 else ap


ALU_ADD = mybir.AluOpType.add


def _runs(labels):
    """group consecutive equal labels -> list of (label, start, len)"""
    out = []
    for i, l in enumerate(labels):
        if out and out[-1][0] == l:
            out[-1][2] += 1
        else:
            out.append([l, i, 1])
    return [tuple(r) for r in out]


def _build():
    global _built
    if _built is not None:
        return _built

    nc = bacc.Bacc("TRN2", target_bir_lowering=False, debug=False,
                   num_devices=NCORES)

    xT = nc.dram_tensor("xT", [DM, TOK], F32, kind="ExternalInput").ap()
    wq = nc.dram_tensor("wq", [DM, 128], F32, kind="ExternalInput").ap()
    wk = nc.dram_tensor("wk", [DM, 128], F32, kind="ExternalInput").ap()
    wv = nc.dram_tensor("wv", [DM, 128], F32, kind="ExternalInput").ap()
    bqkv = nc.dram_tensor("bqkv", [128, 3], F32, kind="ExternalInput").ap()
    g2 = nc.dram_tensor("g2", [128, 512], F32, kind="ExternalInput").ap()
    tcols = nc.dram_tensor("tcols", [128, 2], F32, kind="ExternalInput").ap()
    wo = nc.dram_tensor("wo", [128, DM], F32, kind="ExternalInput").ap()
    partial = nc.dram_tensor("partial", [TOK, DM], F32,
                             kind="ExternalOutput").ap()
    # skew buffer: one [S, 512] f32 region per (b, local h)
    dbuf = nc.dram_tensor("dbuf", [B * HL * S * 512], F32)

    with tile.TileContext(nc) as tc, ExitStack() as ctx:
        singles = ctx.enter_context(tc.tile_pool(name="singles", bufs=1))
        xpool = ctx.enter_context(tc.tile_pool(name="xp", bufs=1))
        projp = ctx.enter_context(tc.tile_pool(name="projp", bufs=2))
        kvarp = ctx.enter_context(tc.tile_pool(name="kvarp", bufs=2))
        vnp = ctx.enter_context(tc.tile_pool(name="vnp", bufs=4))
        expp = ctx.enter_context(tc.tile_pool(name="expp", bufs=4))
        ohp = ctx.enter_context(tc.tile_pool(name="ohp", bufs=2))
        drp = ctx.enter_context(tc.tile_pool(name="drp", bufs=8))
        dsp = ctx.enter_context(tc.tile_pool(name="dsp", bufs=3))
        wop = ctx.enter_context(tc.tile_pool(name="wop", bufs=3))
        recp = ctx.enter_context(tc.tile_pool(name="recp", bufs=4))
        ppsum = ctx.enter_context(tc.tile_pool(name="ppsum", bufs=2, space="PSUM"))
        vpsum = ctx.enter_context(tc.tile_pool(name="vpsum", bufs=1, space="PSUM"))
        spsum = ctx.enter_context(tc.tile_pool(name="spsum", bufs=3, space="PSUM"))
        apsum = ctx.enter_context(tc.tile_pool(name="apsum", bufs=2, space="PSUM"))

        w_sb = {}
        for name, dram in (("wq", wq), ("wk", wk), ("wv", wv)):
            t = singles.tile([128, 8, 128], MM_DT, tag=f"w_{name}")
            nc.sync.dma_start(t[:], _mm(dram.rearrange("(c p) m -> p c m", p=128)))
            w_sb[name] = t
        wo_sb = singles.tile([128, DM], MM_DT, tag="wo")
        nc.sync.dma_start(wo_sb[:], _mm(wo))
        g_sb = singles.tile([128, 512], MM_DT, tag="g")
        nc.sync.dma_start(g_sb[:], _mm(g2))
        tc_sb = singles.tile([128, 2], F32, tag="tc")
        nc.sync.dma_start(tc_sb[:], tcols)
        b_sb = singles.tile([128, 3], F32, tag="b")
        nc.sync.dma_start(b_sb[:], bqkv)
        idb = singles.tile([128, 128], BF16, tag="idb")
        make_identity(nc, idb[:])
        idf = singles.tile([128, 128], F32, tag="idf")
        make_identity(nc, idf[:])

        xT_r = xT.rearrange("(c p) t -> p c t", p=128)  # [128, 8, TOK]

        for b in range(B):
            x_sb = xpool.tile([128, 8, S], MM_DT, tag="x")
            nc.sync.dma_start(x_sb[:], _mm(xT_r[:, :, b * S:(b + 1) * S]))

            qt = projp.tile([128, S], MM_DT, tag="qt")
            kt = projp.tile([128, S], MM_DT, tag="kt")
            vt = projp.tile([128, S], BF16, tag="vt")
            # K + T0 / K + T256 variants (rel term outside the +-128 band)
            k0 = kvarp.tile([128, S], MM_DT, tag="k0")
            k256 = kvarp.tile([128, S], MM_DT, tag="k256")
            for pi, (wname, psb) in enumerate(
                    (("wq", qt), ("wk", kt), ("wv", vt))):
                for nt in range(2):
                    ns = slice(nt * 512, (nt + 1) * 512)
                    ps = ppsum.tile([128, 512], F32, tag="ps")
                    for kc in range(8):
                        nc.tensor.matmul(
                            ps[:],
                            w_sb[wname][:, kc, :],
                            x_sb[:, kc, nt * 512:(nt + 1) * 512],
                            start=(kc == 0), stop=(kc == 7))
                    nc.vector.tensor_scalar_add(
                        psb[:, ns], ps[:], b_sb[:, pi:pi + 1])
                    if wname == "wk":
                        nc.vector.tensor_scalar(
                            k0[:, ns], ps[:], b_sb[:, pi:pi + 1],
                            tc_sb[:, 0:1], ALU_ADD, ALU_ADD)
                        nc.vector.tensor_scalar(
                            k256[:, ns], ps[:], b_sb[:, pi:pi + 1],
                            tc_sb[:, 1:2], ALU_ADD, ALU_ADD)

            # V in natural layout [k, dv] via PE transpose (bf16), + ones col
            vns = []
            for h in range(HL):
                hs = slice(h * 64, (h + 1) * 64)
                vn = vnp.tile([128, 8, 66], BF16, tag="vn")
                nc.vector.memset(vn[:, :, 64:65], 1.0)
                nc.vector.memset(vn[:, :, 65:66], 0.0)
                for k2 in range(8):
                    vps = vpsum.tile([128, 64], BF16, tag="vps")
                    nc.tensor.matmul(
                        vps[:], vt[hs, k2 * 128:(k2 + 1) * 128],
                        idb[hs, hs], is_transpose=True, start=True, stop=True)
                    nc.vector.tensor_copy(vn[:, k2, 0:64], vps[:])
                vns.append(vn)

            ohT = ohp.tile([128, S], MM_DT, tag="oh")

            # --- R phase: D rows = Q . G -> DRAM skew buffer (bf16)
            for q2 in range(8):
                rpss = []
                for h in range(HL):
                    hs = slice(h * 64, (h + 1) * 64)
                    rps = ppsum.tile([128, 512], F32, tag="ps")
                    nc.tensor.matmul(
                        rps[:],
                        qt[hs, q2 * 128:(q2 + 1) * 128],
                        g_sb[hs, :], start=True, stop=True)
                    rpss.append(rps)
                for h in range(HL):
                    bh = b * HL + h
                    dsb = dsp.tile([128, 512], F32, tag="ds")
                    nc.scalar.copy(dsb[:], rpss[h][:])
                    nc.sync.dma_start(
                        bass.AP(tensor=dbuf,
                                offset=bh * S * 512 + q2 * 128 * 512,
                                ap=[[512, 128], [1, 512]]),
                        dsb[:])

            # --- scores (transposed) + softmax + attn@V per q-half
            for qh in range(2):
                apss = [apsum.tile([66, 512], F32, tag="aps", name=f"aps{h}")
                        for h in range(HL)]
                exs = [expp.tile([128, 8, 512], BF16, tag="ex",
                                 name=f"ex{h}")
                       for h in range(HL)]
                def _cls(k2):
                    return ["f256" if (qh * 512 + qs * 128) - k2 * 128 >= 256
                            else ("f0" if (qh * 512 + qs * 128) - k2 * 128
                                  <= -256 else "near")
                            for qs in range(4)]
                k2_order = sorted(range(8),
                                  key=lambda k2: "near" in _cls(k2))
                for k2 in k2_order:
                    cls = _cls(k2)
                    runs = _runs(cls)
                    n_near = sum(1 for c in cls if c == "near")
                    n_mm = len(runs) + n_near
                    spss = []
                    for h in range(HL):
                        hs = slice(h * 64, (h + 1) * 64)
                        sps = spsum.tile([128, 512], F32, tag="sps",
                                         name=f"sps{h}")
                        for mi, (cl, qs0, ln) in enumerate(runs):
                            lhs = {"near": kt, "f0": k0, "f256": k256}[cl]
                            n0 = qs0 * 128
                            nc.tensor.matmul(
                                sps[:, n0:n0 + ln * 128],
                                lhs[hs, k2 * 128:(k2 + 1) * 128],
                                qt[hs, qh * 512 + n0:
                                   qh * 512 + n0 + ln * 128],
                                start=(mi == 0), stop=(mi == n_mm - 1))
                        spss.append(sps)
                    for h in range(HL):
                        bh = b * HL + h
                        mi = len(runs)
                        for qs in range(4):
                            if cls[qs] != "near":
                                continue
                            q0a = qh * 512 + qs * 128
                            off = (bh * S * 512 + q0a * 512
                                   + (k2 * 128 - q0a + 255))
                            dr = drp.tile([128, 128], F32, tag="dr")
                            nc.gpsimd.dma_start(
                                dr[:],
                                bass.AP(tensor=dbuf, offset=off,
                                        ap=[[511, 128], [1, 128]]))
                            nc.tensor.matmul(
                                spss[h][:, qs * 128:(qs + 1) * 128],
                                dr[:], idf[:],
                                is_transpose=True, start=(mi == 0),
                                stop=(mi == n_mm - 1))
                            mi += 1
                    for h in range(HL):
                        nc.scalar.activation(
                            exs[h][:, k2, :], spss[h][:],
                            mybir.ActivationFunctionType.Exp, scale=0.125)
                for h in range(HL):
                    for k2 in range(8):
                        nc.tensor.matmul(
                            apss[h][:], vns[h][:, k2, 0:66],
                            exs[h][:, k2, :],
                            start=(k2 == 0), stop=(k2 == 7))
                for h in range(HL):
                    hs = slice(h * 64, (h + 1) * 64)
                    oc = recp.tile([65, 512], F32, tag="oc")
                    nc.vector.tensor_copy(oc[:], apss[h][0:65, :])
                    rec = recp.tile([1, 512], F32, tag="rec")
                    nc.vector.reciprocal(rec[:], oc[64:65, :])
                    rbc = recp.tile([64, 512], F32, tag="rbc")
                    nc.gpsimd.partition_broadcast(rbc[:], rec[:])
                    nc.vector.tensor_mul(
                        ohT[hs, qh * 512:(qh + 1) * 512],
                        oc[0:64, :], rbc[:])

            # --- output projection (partial: this core's 2 heads only)
            for tt in range(8):
                for nt in range(2):
                    ops = ppsum.tile([128, 512], F32, tag="ps")
                    nc.tensor.matmul(
                        ops[:], ohT[:, tt * 128:(tt + 1) * 128],
                        wo_sb[:, nt * 512:(nt + 1) * 512],
                        start=True, stop=True)
                    osb = wop.tile([128, 512], F32, tag="os")
                    nc.vector.tensor_copy(osb[:], ops[:])
                    nc.sync.dma_start(
                        partial[b * S + tt * 128:b * S + (tt + 1) * 128,
                                nt * 512:(nt + 1) * 512], osb[:])

    nc.compile()
    _built = nc
    return nc


def kernel(x, Wq, bq, Wk, bk, Wv, bv, Wo, bo, rel_table):
    x = np.asarray(x, np.float32)
    Wq = np.asarray(Wq, np.float32)
    Wk = np.asarray(Wk, np.float32)
    Wv = np.asarray(Wv, np.float32)
    Wo = np.asarray(Wo, np.float32)
    bq = np.asarray(bq, np.float32)
    bk = np.asarray(bk, np.float32)
    bv = np.asarray(bv, np.float32)
    bo = np.asarray(bo, np.float32)
    rel_table = np.asarray(rel_table, np.float32)

    xT = np.ascontiguousarray(x.reshape(TOK, DM).T)  # [DM, TOK]
    # G[d, m'] = table[clip(255 - m', -128, 128) + 128, d], m' in [0, 511)
    idx = np.clip(255 - np.arange(511), -MAX_REL, MAX_REL) + MAX_REL
    G = np.concatenate([rel_table[idx].T,
                        np.zeros((DEPTH, 1), np.float32)], axis=1)  # [64, 512]
    g2 = np.ascontiguousarray(np.concatenate([G, G], axis=0))  # [128, 512]
    t0 = np.concatenate([rel_table[0], rel_table[0]])
    t256 = np.concatenate([rel_table[2 * MAX_REL], rel_table[2 * MAX_REL]])
    tcols = np.ascontiguousarray(np.stack([t0, t256], axis=1))  # [128, 2]

    in_maps = []
    for c in range(NCORES):
        sl = slice(c * 128, (c + 1) * 128)
        in_maps.append({
            "xT": xT,
            "wq": np.ascontiguousarray(Wq[:, sl]),
            "wk": np.ascontiguousarray(Wk[:, sl]),
            "wv": np.ascontiguousarray(Wv[:, sl]),
            "bqkv": np.ascontiguousarray(
                np.stack([bq[sl], bk[sl], bv[sl]], axis=1)),
            "g2": g2,
            "tcols": tcols,
            "wo": np.ascontiguousarray(Wo[sl, :]),
        })

    nc = _build()
    res = run_bass_kernel_spmd(nc, in_maps, list(range(NCORES)),
                               trace=TRACE or bool(os.environ.get("KTRACE")))
    LAST["results"] = res
    out = res.results[0]["partial"].astype(np.float64)
    for c in range(1, NCORES):
        out += res.results[c]["partial"]
    out = (out + bo.astype(np.float64)).astype(np.float32)
    return out.reshape(B, S, DM)
